# revision 2
# baseline (speedup 1.0000x reference)
"""Trainium2 Bass kernel for nn_LCADecoderLayer (8-core SPMD, token-parallel).

v3 = v2 + on-device AllGather of roped K / V (own-token projections only,
removing the 4.5x replicated K/V work; KV context is the uniform rank-major
4096-token order with a multiplicative mask so one program serves all cores)
+ hybrid LCA precision (steps 1..7 fp8 DoubleRow, steps 8..9 bf16 for error
margin) + bf16 RoPE.

LCA scaled-state reformulation (exact):
   vhat_t = 576 * w_t / 0.9^(t-1),  rho_t = relu(vhat_t/576) = a_t / 0.9^(t-1)
   vhat_{t+1} = vhat_t + 576*0.9^{-t}*clam - 64*(rho_t G)^T   (64 = 57.6/0.9)
the -64 realized by weight scales (-32 W^T, *1/16, 32 W); diag(G) correction
and clam broadcast ride the same PSUM accumulation. Final h2 = 0.9^9 rho @ W^T
in bf16.
"""

from contextlib import ExitStack

import numpy as np
import ml_dtypes

import concourse.bass as bass
import concourse.mybir as mybir
import concourse.tile as tile
from concourse import bacc
from concourse.bass_utils import run_bass_kernel_spmd
from concourse.masks import make_identity

bf16 = ml_dtypes.bfloat16
fp8 = ml_dtypes.float8_e4m3
F32, BF, F8 = mybir.dt.float32, mybir.dt.bfloat16, mybir.dt.float8e4
AF = mybir.ActivationFunctionType
OP = mybir.AluOpType
DR = mybir.MatmulPerfMode.DoubleRow

P = 128
B, S, D = 2, 2048, 2048
H, HD = 16, 128
DFF, DLCA = 8192, 4096
EPS, LAM = 1e-6, 0.1
NSTEPS = 10
ROPE_THETA = 10000.0

NCORE = 8
CHUNK = S // NCORE            # 256
TOK = 2 * CHUNK               # 512 own tokens / core
KV2 = B * S                   # 4096 kv tokens (uniform rank-major order)
TB = TOK // P                 # 4
DB = D // P                   # 16
RB = DLCA // P                # 32
FB = DFF // P                 # 64
KVB2 = KV2 // P               # 32
ISQD = 1.0 / float(np.sqrt(HD))

SSC = 576.0                   # state scale
A1, SMID, A2 = 32.0, 1.0 / 16.0, 32.0
DEC9 = 0.9 ** (NSTEPS - 1)
BF_LAST = 2                   # last k of the 9 recurrence steps in bf16


# ----------------------------------------------------------------- host prep

def _core_token_map(c):
    b0 = np.arange(256 * c, 256 * c + 256)
    b1 = np.arange(256 * (7 - c), 256 * (8 - c))
    return np.concatenate([b0, b1 + S])


def _rope_tables():
    inv_freq = 1.0 / (ROPE_THETA ** (np.arange(0, HD, 2, dtype=np.float32) / HD))
    t = np.arange(S, dtype=np.float32)
    freqs = np.outer(t, inv_freq)
    emb = np.concatenate([freqs, freqs], -1)           # [S, HD]
    return np.cos(emb).astype(np.float32), np.sin(emb).astype(np.float32)


def _swz(w, nb, cw):
    """[K, X] -> [nb, 128, kb, cw]; result[i,p,j,c] = w[j*128+p, i*cw+c]."""
    K, X = w.shape
    kb = K // P
    assert X == nb * cw
    r = w.reshape(kb, P, nb, cw).transpose(2, 1, 0, 3)
    return np.ascontiguousarray(r)


def _swz_dr(w, nb, cw, scale, dtype):
    """[K, X] -> [nb, 128, kb2, 2, cw] DoubleRow-packed lhsT tiles."""
    K, X = w.shape
    kb2 = K // (2 * P)
    assert X == nb * cw
    r = (w * scale).reshape(kb2, 2, P, nb, cw).transpose(3, 2, 0, 1, 4)
    return np.ascontiguousarray(r).astype(dtype)


# -------------------------------------------------------------- device build

def build_nc():
    nc = bacc.Bacc("TRN2", target_bir_lowering=False, debug=False,
                   num_devices=NCORE)

    def inp(name, shape, dt):
        return nc.dram_tensor(name, list(shape), dt, kind="ExternalInput").ap()

    x_own = inp("x_own", (TOK, D), F32)
    xoT = inp("xoT", (D, TOK), BF)
    maskT = inp("maskT", (P, KVB2, 256), BF)       # multiplicative 1/0, q-half
    cosT = inp("cosT", (HD, TOK), BF)
    sinT = inp("sinT", (HD, TOK), BF)              # rows 0:64 pre-negated
    wq_r = inp("wq_r", (H, P, DB, HD), BF)
    wk_r = inp("wk_r", (H, P, DB, HD), BF)
    wv_g = inp("wv_g", (4, P, DB, 512), BF)
    wo_n = inp("wo_n", (4, P, DB, 512), BF)
    wlcan_r = inp("wlcan_r", (RB, P, DB, P), BF)
    wlcats8 = inp("wlcats8", (DB, P, DB, 2, P), F8)     # -32*W^T, y lhsT
    wlca8 = inp("wlca8", (RB, P, 8, 2, P), F8)          # 32*W, z lhsT
    wlcatsb = inp("wlcatsb", (DB, P, RB, P), BF)        # -32*W^T bf16
    wlcab = inp("wlcab", (RB, P, DB, P), BF)            # 32*W bf16
    gs64 = inp("gs64", (P, RB), F32)                    # 64*diag(W^T W)
    wlcats_n = inp("wlcats_n", (4, P, RB, 512), BF)     # 0.9^9 * W^T
    wg_r = inp("wg_r", (FB, P, DB, HD), BF)
    wu_r = inp("wu_r", (FB, P, DB, HD), BF)
    wd_n = inp("wd_n", (4, 8, P, 8, 512), BF)
    y = nc.dram_tensor("y", [TOK, D], F32, kind="ExternalOutput").ap()

    with tile.TileContext(nc) as tc, ExitStack() as ctx:
        const = ctx.enter_context(tc.tile_pool(name="const", bufs=1))
        ident = const.tile([P, P], BF)
        make_identity(nc, ident)
        ones_col = const.tile([P, 1], BF)
        nc.vector.memset(ones_col[:], 1.0)
        ones_row = const.tile([1, P], F32)
        nc.vector.memset(ones_row[:], 1.0)
        bias_clam = const.tile([P, 1], F32)
        nc.vector.memset(bias_clam[:], -0.1 * LAM)
        bias_vinit = const.tile([P, 1], F32)
        nc.vector.memset(bias_vinit[:], -SSC * LAM)

        dram = ctx.enter_context(tc.tile_pool(name="dram", bufs=1, space="DRAM"))
        kag_in = dram.tile([H, P, TOK], BF)
        kag_out = dram.tile([NCORE * H, P, TOK], BF, addr_space="Shared")
        vag_in = dram.tile([TOK, D], BF)
        vag_out = dram.tile([NCORE * TOK, D], BF, addr_space="Shared")

        hkp_cm = tc.tile_pool(name="hkp", bufs=1, side="left")
        hkp = hkp_cm.__enter__()
        hk = hkp.tile([P, DB, TOK], BF)        # own normed x, transposed
        qTa = hkp.tile([P, H, TOK], BF)        # all roped q heads

        # ---------------- Phase A: own rms scales, hT (all on-chip) ----------------
        with (
            tc.tile_pool(name="pa", bufs=2) as pa,
            tc.tile_pool(name="paps", bufs=2, space="PSUM") as paps,
        ):
            xTs = []
            ps_ss = paps.tile([1, TOK], F32, tag="ps_ss", bufs=1)
            for j in range(DB):
                xT = pa.tile([P, TOK], BF, tag=f"xT{j}")
                nc.sync.dma_start(xT[:], xoT[j * P:(j + 1) * P, :])
                xTs.append(xT)
                x2 = pa.tile([P, TOK], BF, tag="x2")
                nc.vector.tensor_tensor(x2[:], xT[:], xT[:], op=OP.mult)
                nc.tensor.matmul(ps_ss[:], ones_col[:], x2[:],
                                 start=(j == 0), stop=(j == DB - 1))
            t_all = pa.tile([1, TOK], F32, tag="tall")
            nc.vector.tensor_scalar(t_all[:], ps_ss[:], 1.0 / D, EPS,
                                    op0=OP.mult, op1=OP.add)
            r_all = pa.tile([1, TOK], F32, tag="rall")
            nc.vector.reciprocal(r_all[:], t_all[:])
            s_row = pa.tile([1, TOK], F32, tag="srow")
            nc.scalar.activation(s_row[:], r_all[:], AF.Sqrt)
            ps_bc = paps.tile([P, TOK], F32, tag="ps_bc", bufs=1)
            nc.tensor.matmul(ps_bc[:], ones_row[:], s_row[:],
                             start=True, stop=True)
            s_bc = pa.tile([P, TOK], F32, tag="sbc")
            nc.scalar.copy(s_bc[:], ps_bc[:])
            for j in range(DB):
                nc.vector.tensor_tensor(hk[:, j, :], xTs[j][:], s_bc[:],
                                        op=OP.mult)

        # ------- Phase KVP: own k (roped) + v, bounce to DRAM, AllGather -------
        with (
            tc.tile_pool(name="pk", bufs=2) as pk,
            tc.tile_pool(name="pks", bufs=1) as pks,
            tc.tile_pool(name="pkps", bufs=2, space="PSUM") as pkps,
        ):
            cos_sb = pks.tile([P, TOK], BF)
            nc.sync.dma_start(cos_sb[:], cosT[:])
            sin_sb = pks.tile([P, TOK], BF)
            nc.sync.dma_start(sin_sb[:], sinT[:])

            def rope_bf(dst, ps):
                """dst(bf16) = rope(ps). ps is PSUM f32 [P, TOK]."""
                qc = pk.tile([P, TOK], F32, tag="rope_c", name="qc")
                nc.vector.tensor_tensor(qc[:], ps[:], cos_sb[:], op=OP.mult)
                qr = pk.tile([P, TOK], F32, tag="rope_r", name="qr")
                hh2 = HD // 2
                nc.vector.tensor_tensor(qr[:hh2, :], ps[hh2:, :],
                                        sin_sb[:hh2, :], op=OP.mult)
                nc.vector.tensor_tensor(qr[hh2:, :], ps[:hh2, :],
                                        sin_sb[hh2:, :], op=OP.mult)
                nc.vector.tensor_tensor(dst, qc[:], qr[:], op=OP.add)

            for hh in range(H):
                wk_sb = pk.tile([P, DB, HD], BF, tag="wk", name="wk_sb")
                nc.sync.dma_start(wk_sb[:], wk_r[hh])
                ps_k = pkps.tile([P, TOK], F32, tag="ps_k", name="ps_k")
                for j in range(DB):
                    nc.tensor.matmul(ps_k[:], wk_sb[:, j, :], hk[:, j, :],
                                     start=(j == 0), stop=(j == DB - 1))
                kT = pk.tile([P, TOK], BF, tag="kT", name="kT")
                rope_bf(kT[:], ps_k)
                nc.sync.dma_start(kag_in[hh], kT[:])

            nc.gpsimd.collective_compute(
                "AllGather", OP.bypass, ins=[kag_in.opt()],
                outs=[kag_out.opt()], replica_groups=[list(range(NCORE))])

            for g in range(4):
                wv_sb = pk.tile([P, DB, 512], BF, tag="wv", name="wv_sb")
                nc.sync.dma_start(wv_sb[:], wv_g[g])
                for t in range(TB):
                    ps_v = pkps.tile([P, 512], F32, tag="ps_k", name="ps_v")
                    for j in range(DB):
                        nc.tensor.matmul(ps_v[:], hk[:, j, t * P:(t + 1) * P],
                                         wv_sb[:, j, :], start=(j == 0),
                                         stop=(j == DB - 1))
                    vsb = pk.tile([P, 512], BF, tag="vsb", name="vsb")
                    nc.scalar.copy(vsb[:], ps_v[:])
                    nc.sync.dma_start(
                        vag_in[t * P:(t + 1) * P, g * 512:(g + 1) * 512], vsb[:])

            nc.gpsimd.collective_compute(
                "AllGather", OP.bypass, ins=[vag_in.opt()],
                outs=[vag_out.opt()], replica_groups=[list(range(NCORE))])

            # q projections + rope while the collectives fly
            for hh in range(H):
                wq_sb = pk.tile([P, DB, HD], BF, tag="wk", name="wq_sb")
                nc.sync.dma_start(wq_sb[:], wq_r[hh])
                ps_q = pkps.tile([P, TOK], F32, tag="ps_k", name="ps_q")
                for j in range(DB):
                    nc.tensor.matmul(ps_q[:], wq_sb[:, j, :], hk[:, j, :],
                                     start=(j == 0), stop=(j == DB - 1))
                rope_bf(qTa[:, hh, :], ps_q)

        # ---------------- Phase B: attention over gathered KV ----------------
        attp_cm = tc.tile_pool(name="attp", bufs=1, side="right")
        attp = attp_cm.__enter__()
        attnT = attp.tile([P, DB, TOK], BF)

        with (
            tc.tile_pool(name="pb", bufs=1) as pb,
            tc.tile_pool(name="pbs1", bufs=1) as pbs1,
            tc.tile_pool(name="pbs2", bufs=2) as pbs2,
            tc.tile_pool(name="pbps", bufs=2, space="PSUM") as pbps,
        ):
            mk = pb.tile([P, KVB2, 256], BF)
            nc.sync.dma_start(mk[:], maskT[:])
            expT = pb.tile([P, KVB2, 256], BF)
            HQ = [(t % 4) // 2 for t in range(KVB2)]   # q-half per kv block
            FIRST = {0: 0, 1: 2}
            LAST = {0: KVB2 - 3, 1: KVB2 - 1}

            for g in range(4):
                vg = pb.tile([P, KVB2, 512], BF, tag="vg", name="vg")
                nc.sync.dma_start(
                    vg[:], vag_out[:, g * 512:(g + 1) * 512]
                    .rearrange("(t p) c -> p t c", p=P))

                for h4 in range(4):
                    hh = g * 4 + h4
                    kT = pbs2.tile([P, NCORE, TOK], BF, tag="kT2", name="kT2")
                    nc.sync.dma_start(
                        kT[:], kag_out[:].rearrange("(r h) p t -> h p r t", h=H)
                        [hh])

                    for t in range(KVB2):
                        q0 = HQ[t] * 256
                        ps_s = pbps.tile([P, 256], F32, tag="ps_s", name="ps_s")
                        nc.tensor.matmul(ps_s[:],
                                         kT[:, t // TB,
                                            (t % TB) * P:(t % TB + 1) * P],
                                         qTa[:, hh, q0:q0 + 256],
                                         start=True, stop=True)
                        esb = pbs1.tile([P, 256], BF, tag="esb", name="esb")
                        nc.scalar.activation(esb[:], ps_s[:], AF.Exp,
                                             scale=ISQD)
                        nc.vector.tensor_tensor(expT[:, t, :], esb[:],
                                                mk[:, t, :], op=OP.mult)
                    ps_sum = [pbps.tile([1, 256], F32, tag=f"ps_sum{q}",
                                        name=f"ps_sum{q}", bufs=1)
                              for q in (0, 1)]
                    for t in range(KVB2):
                        nc.tensor.matmul(ps_sum[HQ[t]][:], ones_col[:],
                                         expT[:, t, :], start=(t == FIRST[HQ[t]]),
                                         stop=(t == LAST[HQ[t]]))
                    r_row = pbs1.tile([1, TOK], F32, tag="r_row", name="r_row")
                    nc.vector.reciprocal(r_row[:, 0:256], ps_sum[0][:])
                    nc.vector.reciprocal(r_row[:, 256:512], ps_sum[1][:])
                    ps_rbc = pbps.tile([P, TOK], F32, tag="ps_rbc",
                                       name="ps_rbc", bufs=1)
                    nc.tensor.matmul(ps_rbc[:], ones_row[:], r_row[:],
                                     start=True, stop=True)
                    r_bc = pbs1.tile([P, TOK], F32, tag="r_bc", name="r_bc")
                    nc.scalar.copy(r_bc[:], ps_rbc[:])
                    ps_pv = [pbps.tile([P, 256], F32, tag=f"ps_pv{q}",
                                       name=f"ps_pv{q}", bufs=1)
                             for q in (0, 1)]
                    for t in range(KVB2):
                        nc.tensor.matmul(ps_pv[HQ[t]][:],
                                         vg[:, t, h4 * P:(h4 + 1) * P],
                                         expT[:, t, :], start=(t == FIRST[HQ[t]]),
                                         stop=(t == LAST[HQ[t]]))
                    for q in (0, 1):
                        nc.vector.tensor_tensor(
                            attnT[:, hh, q * 256:(q + 1) * 256], ps_pv[q][:],
                            r_bc[:, q * 256:(q + 1) * 256], op=OP.mult)

        hkp_cm.__exit__(None, None, None)
        h1p_cm = tc.tile_pool(name="h1p", bufs=1, side="left")
        h1p = h1p_cm.__enter__()
        h1 = h1p.tile([P, TB, D], F32)

        # ---------------- Phase C: attn @ Wo + residual (m-outer) ----------------
        with (
            tc.tile_pool(name="pc", bufs=2) as pc,
            tc.tile_pool(name="pcw", bufs=4) as pcw,
            tc.tile_pool(name="pcps", bufs=2, space="PSUM") as pcps,
        ):
            wo_sbs = []
            for n in range(4):
                wo_sb = pcw.tile([P, DB, 512], BF, tag="wo", name="wo_sb")
                nc.sync.dma_start(wo_sb[:], wo_n[n])
                wo_sbs.append(wo_sb)
            for m in range(TB):
                for n in range(4):
                    ps_o = pcps.tile([P, 512], F32, tag="ps_o", name="ps_o")
                    for k in range(DB):
                        nc.tensor.matmul(ps_o[:], attnT[:, k, m * P:(m + 1) * P],
                                         wo_sbs[n][:, k, :], start=(k == 0),
                                         stop=(k == DB - 1))
                    xo = pc.tile([P, 512], F32, tag="xo", name="xo")
                    nc.sync.dma_start(
                        xo[:], x_own[m * P:(m + 1) * P, n * 512:(n + 1) * 512])
                    nc.vector.tensor_tensor(h1[:, m, n * 512:(n + 1) * 512],
                                            ps_o[:], xo[:], op=OP.add)

        attp_cm.__exit__(None, None, None)
        hnp_cm = tc.tile_pool(name="hnp", bufs=1, side="right")
        hnp = hnp_cm.__enter__()
        hnT = hnp.tile([P, DB, TOK], BF)

        # ------------- Phase D1: hnT (rmsnorm of h1, transposed) -------------
        with (
            tc.tile_pool(name="pd1s", bufs=1) as pd1s,
            tc.tile_pool(name="pdps", bufs=2, space="PSUM") as pdps,
        ):
            for m in range(TB):
                sq = pd1s.tile([P, D], F32, tag="sq2", name="sq")
                v2 = pd1s.tile([P, 1], F32, tag="v2", name="v2")
                nc.scalar.activation(sq[:], h1[:, m, :], AF.Square,
                                     accum_out=v2[:])
                t2 = pd1s.tile([P, 1], F32, tag="t2", name="t2")
                nc.vector.tensor_scalar(t2[:], v2[:], 1.0 / D, EPS,
                                        op0=OP.mult, op1=OP.add)
                r2 = pd1s.tile([P, 1], F32, tag="r2", name="r2")
                nc.vector.reciprocal(r2[:], t2[:])
                s2 = pd1s.tile([P, 1], F32, tag="s2", name="s2")
                nc.scalar.activation(s2[:], r2[:], AF.Sqrt)
                hn = pd1s.tile([P, D], BF, tag="hn", name="hn")
                nc.vector.tensor_scalar(hn[:], h1[:, m, :], s2[:], None,
                                        op0=OP.mult)
                for j in range(DB):
                    ps_t = pdps.tile([P, P], BF, tag="ps_tr", name="ps_t")
                    nc.tensor.transpose(ps_t[:], hn[:, j * P:(j + 1) * P],
                                        ident[:])
                    nc.scalar.copy(hnT[:, j, m * P:(m + 1) * P], ps_t[:])

        h1p_cm.__exit__(None, None, None)
        wcp_cm = tc.tile_pool(name="wcp", bufs=1, side="left")
        wcp = wcp_cm.__enter__()
        vh = wcp.tile([P, RB, TOK], F32)       # scaled state vhat, 64KB/p
        clamT = wcp.tile([P, RB, TOK], BF)     # 32KB/p
        aT8 = wcp.tile([P, RB, TOK], F8, tag="aTs")
        yT8 = wcp.tile([P, DB, TOK], F8, tag="yTs")
        diag8 = wcp.tile([P, RB, P], F8)       # 4KB/p
        diagb = wcp.tile([P, RB, P], BF)       # 8KB/p
        identc = wcp.tile([P, NSTEPS - 1, P], BF)  # 2.25KB/p

        # ------------- Phase D2: b-proj -> clamT + vh init; diag/identc -------------
        with (
            tc.tile_pool(name="pd3s", bufs=2) as pd3s,
            tc.tile_pool(name="pd3ps", bufs=2, space="PSUM") as pd3ps,
        ):
            for r in range(RB):
                wn_sb = pd3s.tile([P, DB, P], BF, tag="wn", name="wn_sb")
                nc.sync.dma_start(wn_sb[:], wlcan_r[r])
                ps_b = pd3ps.tile([P, TOK], F32, tag="ps_b", name="ps_b")
                for j in range(DB):
                    nc.tensor.matmul(ps_b[:], wn_sb[:, j, :], hnT[:, j, :],
                                     start=(j == 0), stop=(j == DB - 1))
                nc.scalar.activation(clamT[:, r, :], ps_b[:], AF.Identity,
                                     scale=0.1, bias=bias_clam[:])
                nc.scalar.activation(vh[:, r, :], ps_b[:], AF.Identity,
                                     scale=0.1 * SSC, bias=bias_vinit[:])
            gst = pd3s.tile([P, RB], F32, tag="gst", name="gst")
            nc.sync.dma_start(gst[:], gs64[:])
            for r in range(RB):
                nc.vector.tensor_scalar(diag8[:, r, :], ident[:],
                                        gst[:, r:r + 1], None, op0=OP.mult)
                nc.vector.tensor_scalar(diagb[:, r, :], ident[:],
                                        gst[:, r:r + 1], None, op0=OP.mult)
            for t in range(1, NSTEPS):
                nc.vector.tensor_scalar(identc[:, t - 1, :], ident[:],
                                        SSC * 0.9 ** (-t), None, op0=OP.mult)

        hnp_cm.__exit__(None, None, None)

        # ---------------- Phase E: LCA recurrence ----------------
        NFP8 = NSTEPS - 1 - BF_LAST
        with (
            tc.tile_pool(name="pe", bufs=3) as pe,
            tc.tile_pool(name="peb", bufs=2) as peb,
            tc.tile_pool(name="peps", bufs=2, space="PSUM") as peps,
        ):
            K2Y = DB            # 16 pair-tiles contracting DLCA
            K2Z = DB // 2       # 8 pair-tiles contracting D

            for t in range(1, NSTEPS):
                if t <= NFP8:
                    for r in range(RB):
                        nc.scalar.activation(aT8[:, r, :], vh[:, r, :], AF.Relu,
                                             scale=1.0 / SSC)
                    for d in range(DB):
                        w1_sb = pe.tile([P, K2Y, 2, P], F8, tag="w1",
                                        name="w1_sb")
                        nc.sync.dma_start(w1_sb[:], wlcats8[d])
                        ps_y = peps.tile([P, TOK], F32, tag="ps_y", name="ps_y")
                        for k2 in range(K2Y):
                            nc.tensor.matmul(ps_y[:], w1_sb[:, k2, :, :],
                                             aT8[:, 2 * k2:2 * k2 + 2, :],
                                             start=(k2 == 0),
                                             stop=(k2 == K2Y - 1), perf_mode=DR)
                        nc.scalar.activation(yT8[:, d, :], ps_y[:], AF.Identity,
                                             scale=SMID)
                    for r in range(RB):
                        w2_sb = pe.tile([P, K2Z, 2, P], F8, tag="w2s",
                                        name="w2_sb")
                        nc.sync.dma_start(w2_sb[:], wlca8[r])
                        ps_z = peps.tile([P, TOK], F32, tag="ps_z", name="ps_z")
                        for j2 in range(K2Z):
                            nc.tensor.matmul(ps_z[:], w2_sb[:, j2, :, :],
                                             yT8[:, 2 * j2:2 * j2 + 2, :],
                                             start=(j2 == 0), stop=False,
                                             perf_mode=DR)
                        nc.tensor.matmul(ps_z[:], diag8[:, r, :], aT8[:, r, :],
                                         start=False, stop=False)
                        nc.tensor.matmul(ps_z[:], identc[:, t - 1, :],
                                         clamT[:, r, :], start=False, stop=True)
                        nc.vector.tensor_tensor(vh[:, r, :], vh[:, r, :],
                                                ps_z[:], op=OP.add)
                else:
                    aTb = wcp.tile([P, RB, TOK], BF, tag="aTs", name="aTb")
                    yTb = wcp.tile([P, DB, TOK], BF, tag="yTs", name="yTb")
                    for r in range(RB):
                        nc.scalar.activation(aTb[:, r, :], vh[:, r, :], AF.Relu,
                                             scale=1.0 / SSC)
                    for d in range(DB):
                        w1b = peb.tile([P, RB, P], BF, tag="w1b", name="w1b")
                        nc.sync.dma_start(w1b[:], wlcatsb[d])
                        ps_y = peps.tile([P, TOK], F32, tag="ps_y", name="ps_y")
                        for k in range(RB):
                            nc.tensor.matmul(ps_y[:], w1b[:, k, :],
                                             aTb[:, k, :], start=(k == 0),
                                             stop=(k == RB - 1))
                        nc.scalar.activation(yTb[:, d, :], ps_y[:], AF.Identity,
                                             scale=SMID)
                    for r in range(RB):
                        w2b = peb.tile([P, DB, P], BF, tag="w2b", name="w2b")
                        nc.sync.dma_start(w2b[:], wlcab[r])
                        ps_z = peps.tile([P, TOK], F32, tag="ps_z", name="ps_z")
                        for j in range(DB):
                            nc.tensor.matmul(ps_z[:], w2b[:, j, :], yTb[:, j, :],
                                             start=(j == 0), stop=False)
                        nc.tensor.matmul(ps_z[:], diagb[:, r, :], aTb[:, r, :],
                                         start=False, stop=False)
                        nc.tensor.matmul(ps_z[:], identc[:, t - 1, :],
                                         clamT[:, r, :], start=False, stop=True)
                        nc.vector.tensor_tensor(vh[:, r, :], vh[:, r, :],
                                                ps_z[:], op=OP.add)

        # final rho in bf16 for the output projection
        atp_cm = tc.tile_pool(name="atp", bufs=1, side="right")
        atp = atp_cm.__enter__()
        aTf = atp.tile([P, RB, TOK], BF)
        for r in range(RB):
            nc.scalar.activation(aTf[:, r, :], vh[:, r, :], AF.Relu,
                                 scale=1.0 / SSC)

        wcp_cm.__exit__(None, None, None)
        h2p_cm = tc.tile_pool(name="h2p", bufs=1, side="left")
        h2p = h2p_cm.__enter__()
        h2 = h2p.tile([P, TB, D], F32)

        # ---------------- Phase F: h2 = 0.9^9 * rho @ W_lca^T ----------------
        with (
            tc.tile_pool(name="pf", bufs=2) as pf,
            tc.tile_pool(name="pfps", bufs=2, space="PSUM") as pfps,
        ):
            for n in range(4):
                wt_sb = pf.tile([P, RB, 512], BF, tag="wts", name="wt_sb")
                nc.sync.dma_start(wt_sb[:], wlcats_n[n])
                for m in range(TB):
                    ps_h = pfps.tile([P, 512], F32, tag="ps_h", name="ps_h")
                    for k in range(RB):
                        nc.tensor.matmul(ps_h[:], aTf[:, k, m * P:(m + 1) * P],
                                         wt_sb[:, k, :], start=(k == 0),
                                         stop=(k == RB - 1))
                    nc.scalar.copy(h2[:, m, n * 512:(n + 1) * 512], ps_h[:])

        atp_cm.__exit__(None, None, None)

        # ---------------- Phase G: MLP ----------------
        with (
            tc.tile_pool(name="pg", bufs=1, side="right") as pg,
            tc.tile_pool(name="pgs1", bufs=1) as pgs1,
            tc.tile_pool(name="pgs", bufs=2) as pgs,
            tc.tile_pool(name="pgps", bufs=2, space="PSUM") as pgps,
            tc.tile_pool(name="pgpd", bufs=1, space="PSUM") as pgpd,
        ):
            prodT = pg.tile([P, FB, TOK], BF)      # 64KB/p
            mT = pg.tile([P, DB, TOK], BF)
            for m in range(TB):
                sq = pgs1.tile([P, D], F32, tag="sq3", name="sq")
                v3 = pgs1.tile([P, 1], F32, tag="v3", name="v3")
                nc.scalar.activation(sq[:], h2[:, m, :], AF.Square,
                                     accum_out=v3[:])
                t3 = pgs1.tile([P, 1], F32, tag="t3", name="t3")
                nc.vector.tensor_scalar(t3[:], v3[:], 1.0 / D, EPS,
                                        op0=OP.mult, op1=OP.add)
                r3 = pgs1.tile([P, 1], F32, tag="r3", name="r3")
                nc.vector.reciprocal(r3[:], t3[:])
                s3 = pgs1.tile([P, 1], F32, tag="s3", name="s3")
                nc.scalar.activation(s3[:], r3[:], AF.Sqrt)
                mb = pgs1.tile([P, D], BF, tag="mb", name="mb")
                nc.vector.tensor_scalar(mb[:], h2[:, m, :], s3[:], None,
                                        op0=OP.mult)
                for j in range(DB):
                    ps_t = pgps.tile([P, P], BF, tag="ps_tr3", name="ps_t")
                    nc.tensor.transpose(ps_t[:], mb[:, j * P:(j + 1) * P],
                                        ident[:])
                    nc.scalar.copy(mT[:, j, m * P:(m + 1) * P], ps_t[:])

            for f in range(FB):
                wgs = pgs.tile([P, DB, HD], BF, tag="wgs", name="wgs")
                nc.sync.dma_start(wgs[:], wg_r[f])
                ps_g = pgps.tile([P, TOK], F32, tag="ps_g", name="ps_g")
                for j in range(DB):
                    nc.tensor.matmul(ps_g[:], wgs[:, j, :], mT[:, j, :],
                                     start=(j == 0), stop=(j == DB - 1))
                gT = pgs.tile([P, TOK], BF, tag="gT", name="gT")
                nc.scalar.activation(gT[:], ps_g[:], AF.Silu)
                wus = pgs.tile([P, DB, HD], BF, tag="wus", name="wus")
                nc.sync.dma_start(wus[:], wu_r[f])
                ps_u = pgps.tile([P, TOK], F32, tag="ps_g", name="ps_u")
                for j in range(DB):
                    nc.tensor.matmul(ps_u[:], wus[:, j, :], mT[:, j, :],
                                     start=(j == 0), stop=(j == DB - 1))
                nc.vector.tensor_tensor(prodT[:, f, :], ps_u[:], gT[:],
                                        op=OP.mult)

            for n in range(4):
                ps_d = [pgpd.tile([P, 512], F32, tag=f"ps_d{m}",
                                  name=f"ps_d{m}")
                        for m in range(TB)]
                for kg in range(8):
                    wds = pgs.tile([P, 8, 512], BF, tag="wds", name="wds")
                    nc.sync.dma_start(wds[:], wd_n[n][kg])
                    for m in range(TB):
                        for k in range(8):
                            kk = kg * 8 + k
                            nc.tensor.matmul(
                                ps_d[m][:], prodT[:, kk, m * P:(m + 1) * P],
                                wds[:, k, :], start=(kg == 0 and k == 0),
                                stop=(kg == 7 and k == 7))
                for m in range(TB):
                    yo = pgs.tile([P, 512], F32, tag="yo", name="yo")
                    nc.vector.tensor_tensor(yo[:], ps_d[m][:],
                                            h2[:, m, n * 512:(n + 1) * 512],
                                            op=OP.add)
                    nc.sync.dma_start(
                        y[m * P:(m + 1) * P, n * 512:(n + 1) * 512], yo[:])

        h2p_cm.__exit__(None, None, None)

    nc.compile()
    return nc


_NC_CACHE = None


def _get_nc():
    global _NC_CACHE
    if _NC_CACHE is None:
        _NC_CACHE = build_nc()
    return _NC_CACHE


def _prep_weights(inputs):
    f32 = np.float32
    wln_in = np.asarray(inputs["w_ln_in"], f32)
    wln_lca = np.asarray(inputs["w_ln_lca"], f32)
    wln_post = np.asarray(inputs["w_ln_post"], f32)
    Wq = np.asarray(inputs["Wq"], f32) * wln_in[:, None]
    Wk = np.asarray(inputs["Wk"], f32) * wln_in[:, None]
    Wv = np.asarray(inputs["Wv"], f32) * wln_in[:, None]
    Wo = np.asarray(inputs["Wo"], f32)
    Wlca = np.asarray(inputs["W_lca"], f32)
    Wlca_n = Wlca * wln_lca[:, None]
    Wg = np.asarray(inputs["W_gate"], f32) * wln_post[:, None]
    Wu = np.asarray(inputs["W_up"], f32) * wln_post[:, None]
    Wd = np.asarray(inputs["W_down"], f32)
    WlcaT = np.ascontiguousarray(Wlca.T)
    gs = np.einsum("ij,ij->j", Wlca, Wlca)
    gs64 = np.ascontiguousarray((64.0 * gs).reshape(RB, P).T).astype(f32)
    c = lambda a: np.ascontiguousarray(a).astype(bf16)
    return {
        "wq_r": c(_swz(Wq, H, HD)), "wk_r": c(_swz(Wk, H, HD)),
        "wv_g": c(_swz(Wv, 4, 512)), "wo_n": c(_swz(Wo, 4, 512)),
        "wlcan_r": c(_swz(Wlca_n, RB, P)),
        "wlcats8": _swz_dr(WlcaT, DB, P, -A1, fp8),
        "wlca8": _swz_dr(Wlca, RB, P, A2, fp8),
        "wlcatsb": c(_swz(WlcaT * (-A1), DB, P)),
        "wlcab": c(_swz(Wlca * A2, RB, P)),
        "gs64": gs64,
        "wlcats_n": c(_swz(WlcaT * DEC9, 4, 512)),
        "wg_r": c(_swz(Wg, FB, HD)), "wu_r": c(_swz(Wu, FB, HD)),
        "wd_n": np.ascontiguousarray(
            c(_swz(Wd, 4, 512)).reshape(4, P, 8, 8, 512)
            .transpose(0, 2, 1, 3, 4)),
    }


def make_in_maps(inputs):
    hs = np.asarray(inputs["hidden_states"], np.float32).reshape(B * S, D)
    wmaps = _prep_weights(inputs)
    cos, sin = _rope_tables()
    all_pos = np.concatenate([_core_token_map(r) for r in range(NCORE)])
    kv_pos, kv_batch = all_pos % S, all_pos // S
    in_maps, owns = [], []
    for cix in range(NCORE):
        own = _core_token_map(cix)
        x_own = np.ascontiguousarray(hs[own])
        xoT = np.ascontiguousarray(x_own.T).astype(bf16)
        q_pos, q_batch = own % S, own // S
        vis = (kv_batch[:, None] == q_batch[None, :]) & (
            kv_pos[:, None] <= q_pos[None, :])
        vis = vis.astype(np.float32).reshape(KVB2, P, TOK)
        mhalf = np.zeros((KVB2, P, 256), np.float32)
        for t in range(KVB2):
            q0 = ((t % 4) // 2) * 256
            mhalf[t] = vis[t, :, q0:q0 + 256]
        maskT = np.ascontiguousarray(mhalf.transpose(1, 0, 2)).astype(bf16)
        cosT = np.ascontiguousarray(cos[q_pos].T).astype(bf16)
        sinT = np.ascontiguousarray(sin[q_pos].T)
        sinT[:HD // 2] *= -1.0
        sinT = sinT.astype(bf16)
        m = {
            "x_own": x_own, "xoT": xoT,
            "maskT": maskT, "cosT": cosT, "sinT": sinT, **wmaps,
        }
        in_maps.append(m)
        owns.append(own)
    return in_maps, owns


def kernel(**inputs) -> np.ndarray:
    nc = _get_nc()
    in_maps, owns = make_in_maps(inputs)
    res = run_bass_kernel_spmd(nc, in_maps, core_ids=list(range(NCORE)))
    out = np.zeros((B * S, D), np.float32)
    for cix in range(NCORE):
        out[owns[cix]] = res.results[cix]["y"]
    return out.reshape(B, S, D)


# revision 4
# speedup vs baseline: 1.1892x; 1.1892x over previous
"""Trainium2 Bass kernel for nn_LCADecoderLayer (8-core SPMD, token-parallel).

v3 = v2 + on-device AllGather of roped K / V (own-token projections only,
removing the 4.5x replicated K/V work; KV context is the uniform rank-major
4096-token order with a multiplicative mask so one program serves all cores)
+ hybrid LCA precision (steps 1..7 fp8 DoubleRow, steps 8..9 bf16 for error
margin) + bf16 RoPE.

LCA scaled-state reformulation (exact):
   vhat_t = 576 * w_t / 0.9^(t-1),  rho_t = relu(vhat_t/576) = a_t / 0.9^(t-1)
   vhat_{t+1} = vhat_t + 576*0.9^{-t}*clam - 64*(rho_t G)^T   (64 = 57.6/0.9)
the -64 realized by weight scales (-32 W^T, *1/16, 32 W); diag(G) correction
and clam broadcast ride the same PSUM accumulation. Final h2 = 0.9^9 rho @ W^T
in bf16.
"""

from contextlib import ExitStack

import numpy as np
import ml_dtypes

import concourse.bass as bass
import concourse.mybir as mybir
import concourse.tile as tile
from concourse import bacc
from concourse.bass_utils import run_bass_kernel_spmd
from concourse.masks import make_identity

bf16 = ml_dtypes.bfloat16
fp8 = ml_dtypes.float8_e4m3
F32, BF, F8 = mybir.dt.float32, mybir.dt.bfloat16, mybir.dt.float8e4
AF = mybir.ActivationFunctionType
OP = mybir.AluOpType
DR = mybir.MatmulPerfMode.DoubleRow

P = 128
B, S, D = 2, 2048, 2048
H, HD = 16, 128
DFF, DLCA = 8192, 4096
EPS, LAM = 1e-6, 0.1
NSTEPS = 10
ROPE_THETA = 10000.0

NCORE = 8
CHUNK = S // NCORE            # 256
TOK = 2 * CHUNK               # 512 own tokens / core
KV2 = B * S                   # 4096 kv tokens (uniform rank-major order)
TB = TOK // P                 # 4
DB = D // P                   # 16
RB = DLCA // P                # 32
FB = DFF // P                 # 64
KVB2 = KV2 // P               # 32
ISQD = 1.0 / float(np.sqrt(HD))

SSC = 576.0                   # state scale
A1, SMID, A2 = 32.0, 1.0 / 16.0, 32.0
DEC9 = 0.9 ** (NSTEPS - 1)
BF_LAST = 2                   # last k of the 9 recurrence steps in bf16


# ----------------------------------------------------------------- host prep

def _core_token_map(c):
    b0 = np.arange(256 * c, 256 * c + 256)
    b1 = np.arange(256 * (7 - c), 256 * (8 - c))
    return np.concatenate([b0, b1 + S])


def _rope_tables():
    inv_freq = 1.0 / (ROPE_THETA ** (np.arange(0, HD, 2, dtype=np.float32) / HD))
    t = np.arange(S, dtype=np.float32)
    freqs = np.outer(t, inv_freq)
    emb = np.concatenate([freqs, freqs], -1)           # [S, HD]
    return np.cos(emb).astype(np.float32), np.sin(emb).astype(np.float32)


def _swz(w, nb, cw):
    """[K, X] -> [nb, 128, kb, cw]; result[i,p,j,c] = w[j*128+p, i*cw+c]."""
    K, X = w.shape
    kb = K // P
    assert X == nb * cw
    r = w.reshape(kb, P, nb, cw).transpose(2, 1, 0, 3)
    return np.ascontiguousarray(r)


def _swz_dr(w, nb, cw, scale, dtype):
    """[K, X] -> [nb, 128, kb2, 2, cw] DoubleRow-packed lhsT tiles."""
    K, X = w.shape
    kb2 = K // (2 * P)
    assert X == nb * cw
    r = (w * scale).reshape(kb2, 2, P, nb, cw).transpose(3, 2, 0, 1, 4)
    return np.ascontiguousarray(r).astype(dtype)


# -------------------------------------------------------------- device build

def build_nc():
    nc = bacc.Bacc("TRN2", target_bir_lowering=False, debug=False,
                   num_devices=NCORE)

    def inp(name, shape, dt):
        return nc.dram_tensor(name, list(shape), dt, kind="ExternalInput").ap()

    x_own = inp("x_own", (TOK, D), F32)
    xoT = inp("xoT", (D, TOK), BF)
    maskT = inp("maskT", (P, KVB2, 256), BF)       # multiplicative 1/0, q-half
    cosT = inp("cosT", (HD, TOK), BF)
    sinT = inp("sinT", (HD, TOK), BF)              # rows 0:64 pre-negated
    wq_r = inp("wq_r", (H, P, DB, HD), BF)
    wk_r = inp("wk_r", (H, P, DB, HD), BF)
    wv_g = inp("wv_g", (4, P, DB, 512), BF)
    wo_n = inp("wo_n", (4, P, DB, 512), BF)
    wlcan_r = inp("wlcan_r", (RB, P, DB, P), BF)
    wlcats8 = inp("wlcats8", (DB, P, DB, 2, P), F8)     # -32*W^T, y lhsT
    wlca8 = inp("wlca8", (RB, P, 8, 2, P), F8)          # 32*W, z lhsT
    wlcatsb = inp("wlcatsb", (DB, P, RB, P), BF)        # -32*W^T bf16
    wlcab = inp("wlcab", (RB, P, DB, P), BF)            # 32*W bf16
    gs64 = inp("gs64", (P, RB), F32)                    # 64*diag(W^T W)
    wlcats_n = inp("wlcats_n", (4, P, RB, 512), BF)     # 0.9^9 * W^T
    wg_r = inp("wg_r", (FB, P, DB, HD), BF)
    wu_r = inp("wu_r", (FB, P, DB, HD), BF)
    wd_n = inp("wd_n", (4, 8, P, 8, 512), BF)
    y = nc.dram_tensor("y", [TOK, D], F32, kind="ExternalOutput").ap()

    with tile.TileContext(nc) as tc, ExitStack() as ctx:
        const = ctx.enter_context(tc.tile_pool(name="const", bufs=1))
        ident = const.tile([P, P], BF)
        make_identity(nc, ident)
        ones_col = const.tile([P, 1], BF)
        nc.vector.memset(ones_col[:], 1.0)
        ones_row = const.tile([1, P], F32)
        nc.vector.memset(ones_row[:], 1.0)
        bias_clam = const.tile([P, 1], F32)
        nc.vector.memset(bias_clam[:], -0.1 * LAM)
        bias_vinit = const.tile([P, 1], F32)
        nc.vector.memset(bias_vinit[:], -SSC * LAM)

        dram = ctx.enter_context(tc.tile_pool(name="dram", bufs=1, space="DRAM"))
        kag_in = dram.tile([H, P, TOK], BF)
        kag_out = dram.tile([NCORE * H, P, TOK], BF, addr_space="Shared")
        vag_in = dram.tile([TOK, D], BF)
        vag_out = dram.tile([NCORE * TOK, D], BF, addr_space="Shared")

        hkp_cm = tc.tile_pool(name="hkp", bufs=1, side="left")
        hkp = hkp_cm.__enter__()
        hk = hkp.tile([P, DB, TOK], BF)        # own normed x, transposed
        qTa = hkp.tile([P, H, TOK], BF)        # all roped q heads

        # ---------------- Phase A: own rms scales, hT (all on-chip) ----------------
        with (
            tc.tile_pool(name="pa", bufs=2) as pa,
            tc.tile_pool(name="paps", bufs=2, space="PSUM") as paps,
        ):
            xTs = []
            ps_ss = paps.tile([1, TOK], F32, tag="ps_ss", bufs=1)
            for j in range(DB):
                xT = pa.tile([P, TOK], BF, tag=f"xT{j}")
                nc.sync.dma_start(xT[:], xoT[j * P:(j + 1) * P, :])
                xTs.append(xT)
                x2 = pa.tile([P, TOK], BF, tag="x2")
                nc.vector.tensor_tensor(x2[:], xT[:], xT[:], op=OP.mult)
                nc.tensor.matmul(ps_ss[:], ones_col[:], x2[:],
                                 start=(j == 0), stop=(j == DB - 1))
            t_all = pa.tile([1, TOK], F32, tag="tall")
            nc.vector.tensor_scalar(t_all[:], ps_ss[:], 1.0 / D, EPS,
                                    op0=OP.mult, op1=OP.add)
            r_all = pa.tile([1, TOK], F32, tag="rall")
            nc.vector.reciprocal(r_all[:], t_all[:])
            s_row = pa.tile([1, TOK], F32, tag="srow")
            nc.scalar.activation(s_row[:], r_all[:], AF.Sqrt)
            ps_bc = paps.tile([P, TOK], F32, tag="ps_bc", bufs=1)
            nc.tensor.matmul(ps_bc[:], ones_row[:], s_row[:],
                             start=True, stop=True)
            s_bc = pa.tile([P, TOK], F32, tag="sbc")
            nc.scalar.copy(s_bc[:], ps_bc[:])
            for j in range(DB):
                nc.vector.tensor_tensor(hk[:, j, :], xTs[j][:], s_bc[:],
                                        op=OP.mult)

        # ------- Phase KVP: own k (roped) + v, bounce to DRAM, AllGather -------
        with (
            tc.tile_pool(name="pk", bufs=2) as pk,
            tc.tile_pool(name="pks", bufs=1) as pks,
            tc.tile_pool(name="pkps", bufs=2, space="PSUM") as pkps,
        ):
            cos_sb = pks.tile([P, TOK], BF)
            nc.sync.dma_start(cos_sb[:], cosT[:])
            sin_sb = pks.tile([P, TOK], BF)
            nc.sync.dma_start(sin_sb[:], sinT[:])

            def rope_bf(dst, ps):
                """dst(bf16) = rope(ps). ps is PSUM f32 [P, TOK]."""
                qc = pk.tile([P, TOK], F32, tag="rope_c", name="qc")
                nc.vector.tensor_tensor(qc[:], ps[:], cos_sb[:], op=OP.mult)
                qr = pk.tile([P, TOK], F32, tag="rope_r", name="qr")
                hh2 = HD // 2
                nc.vector.tensor_tensor(qr[:hh2, :], ps[hh2:, :],
                                        sin_sb[:hh2, :], op=OP.mult)
                nc.vector.tensor_tensor(qr[hh2:, :], ps[:hh2, :],
                                        sin_sb[hh2:, :], op=OP.mult)
                nc.vector.tensor_tensor(dst, qc[:], qr[:], op=OP.add)

            for hh in range(H):
                wk_sb = pk.tile([P, DB, HD], BF, tag="wk", name="wk_sb")
                nc.sync.dma_start(wk_sb[:], wk_r[hh])
                ps_k = pkps.tile([P, TOK], F32, tag="ps_k", name="ps_k")
                for j in range(DB):
                    nc.tensor.matmul(ps_k[:], wk_sb[:, j, :], hk[:, j, :],
                                     start=(j == 0), stop=(j == DB - 1))
                kT = pk.tile([P, TOK], BF, tag="kT", name="kT")
                rope_bf(kT[:], ps_k)
                nc.sync.dma_start(kag_in[hh], kT[:])

            nc.gpsimd.collective_compute(
                "AllGather", OP.bypass, ins=[kag_in.opt()],
                outs=[kag_out.opt()], replica_groups=[list(range(NCORE))])

            for g in range(4):
                wv_sb = pk.tile([P, DB, 512], BF, tag="wv", name="wv_sb")
                nc.sync.dma_start(wv_sb[:], wv_g[g])
                for t in range(TB):
                    ps_v = pkps.tile([P, 512], F32, tag="ps_k", name="ps_v")
                    for j in range(DB):
                        nc.tensor.matmul(ps_v[:], hk[:, j, t * P:(t + 1) * P],
                                         wv_sb[:, j, :], start=(j == 0),
                                         stop=(j == DB - 1))
                    vsb = pk.tile([P, 512], BF, tag="vsb", name="vsb")
                    nc.scalar.copy(vsb[:], ps_v[:])
                    nc.sync.dma_start(
                        vag_in[t * P:(t + 1) * P, g * 512:(g + 1) * 512], vsb[:])

            nc.gpsimd.collective_compute(
                "AllGather", OP.bypass, ins=[vag_in.opt()],
                outs=[vag_out.opt()], replica_groups=[list(range(NCORE))])

            # q projections + rope while the collectives fly
            for hh in range(H):
                wq_sb = pk.tile([P, DB, HD], BF, tag="wk", name="wq_sb")
                nc.sync.dma_start(wq_sb[:], wq_r[hh])
                ps_q = pkps.tile([P, TOK], F32, tag="ps_k", name="ps_q")
                for j in range(DB):
                    nc.tensor.matmul(ps_q[:], wq_sb[:, j, :], hk[:, j, :],
                                     start=(j == 0), stop=(j == DB - 1))
                rope_bf(qTa[:, hh, :], ps_q)

        # ---------------- Phase B: attention over gathered KV ----------------
        attp_cm = tc.tile_pool(name="attp", bufs=1, side="right")
        attp = attp_cm.__enter__()
        attnT = attp.tile([P, DB, TOK], BF)

        with (
            tc.tile_pool(name="pb", bufs=1) as pb,
            tc.tile_pool(name="pbs1", bufs=1) as pbs1,
            tc.tile_pool(name="pbs2", bufs=2) as pbs2,
            tc.tile_pool(name="pbps", bufs=2, space="PSUM") as pbps,
        ):
            mk = pb.tile([P, KVB2, 256], BF)
            nc.sync.dma_start(mk[:], maskT[:])
            HQ = [(t % 4) // 2 for t in range(KVB2)]   # q-half per kv block
            FIRST = {0: 0, 1: 2}
            LAST = {0: KVB2 - 3, 1: KVB2 - 1}

            for g in range(4):
                vg = pbs2.tile([P, KVB2, 512], BF, tag="vg", name="vg")
                nc.sync.dma_start(
                    vg[:], vag_out[:, g * 512:(g + 1) * 512]
                    .rearrange("(t p) c -> p t c", p=P))

                for h4 in range(4):
                    hh = g * 4 + h4
                    expT = pbs2.tile([P, KVB2, 256], BF, tag="expT",
                                     name="expT")
                    kT = pbs2.tile([P, NCORE, TOK], BF, tag="kT2", name="kT2")
                    nc.sync.dma_start(
                        kT[:], kag_out[:].rearrange("(r h) p t -> h p r t", h=H)
                        [hh])

                    for t in range(KVB2):
                        q0 = HQ[t] * 256
                        ps_s = pbps.tile([P, 256], F32, tag="ps_s", name="ps_s",
                                         bufs=3)
                        nc.tensor.matmul(ps_s[:],
                                         kT[:, t // TB,
                                            (t % TB) * P:(t % TB + 1) * P],
                                         qTa[:, hh, q0:q0 + 256],
                                         start=True, stop=True)
                        esb = pbs1.tile([P, 256], BF, tag="esb", name="esb",
                                        bufs=3)
                        nc.scalar.activation(esb[:], ps_s[:], AF.Exp,
                                             scale=ISQD)
                        nc.vector.tensor_tensor(expT[:, t, :], esb[:],
                                                mk[:, t, :], op=OP.mult)
                    ps_sum = [pbps.tile([1, 256], F32, tag=f"ps_sum{q}",
                                        name=f"ps_sum{q}", bufs=1)
                              for q in (0, 1)]
                    for t in range(KVB2):
                        nc.tensor.matmul(ps_sum[HQ[t]][:], ones_col[:],
                                         expT[:, t, :], start=(t == FIRST[HQ[t]]),
                                         stop=(t == LAST[HQ[t]]))
                    r_row = pbs1.tile([1, TOK], F32, tag="r_row", name="r_row")
                    nc.vector.reciprocal(r_row[:, 0:256], ps_sum[0][:])
                    nc.vector.reciprocal(r_row[:, 256:512], ps_sum[1][:])
                    ps_rbc = pbps.tile([P, TOK], F32, tag="ps_rbc",
                                       name="ps_rbc", bufs=1)
                    nc.tensor.matmul(ps_rbc[:], ones_row[:], r_row[:],
                                     start=True, stop=True)
                    r_bc = pbs1.tile([P, TOK], F32, tag="r_bc", name="r_bc")
                    nc.scalar.copy(r_bc[:], ps_rbc[:])
                    ps_pv = [pbps.tile([P, 256], F32, tag=f"ps_pv{q}",
                                       name=f"ps_pv{q}", bufs=1)
                             for q in (0, 1)]
                    for t in range(KVB2):
                        nc.tensor.matmul(ps_pv[HQ[t]][:],
                                         vg[:, t, h4 * P:(h4 + 1) * P],
                                         expT[:, t, :], start=(t == FIRST[HQ[t]]),
                                         stop=(t == LAST[HQ[t]]))
                    for q in (0, 1):
                        nc.vector.tensor_tensor(
                            attnT[:, hh, q * 256:(q + 1) * 256], ps_pv[q][:],
                            r_bc[:, q * 256:(q + 1) * 256], op=OP.mult)

        hkp_cm.__exit__(None, None, None)
        h1p_cm = tc.tile_pool(name="h1p", bufs=1, side="left")
        h1p = h1p_cm.__enter__()
        h1 = h1p.tile([P, TB, D], F32)
        v2acc = h1p.tile([P, TB, 4], F32)

        # ---------------- Phase C: attn @ Wo + residual (m-outer) ----------------
        with (
            tc.tile_pool(name="pc", bufs=2) as pc,
            tc.tile_pool(name="pcw", bufs=4) as pcw,
            tc.tile_pool(name="pcps", bufs=2, space="PSUM") as pcps,
        ):
            wo_sbs = []
            for n in range(4):
                wo_sb = pcw.tile([P, DB, 512], BF, tag="wo", name="wo_sb")
                nc.sync.dma_start(wo_sb[:], wo_n[n])
                wo_sbs.append(wo_sb)
            for m in range(TB):
                for n in range(4):
                    ps_o = pcps.tile([P, 512], F32, tag="ps_o", name="ps_o")
                    for k in range(DB):
                        nc.tensor.matmul(ps_o[:], attnT[:, k, m * P:(m + 1) * P],
                                         wo_sbs[n][:, k, :], start=(k == 0),
                                         stop=(k == DB - 1))
                    xo = pc.tile([P, 512], F32, tag="xo", name="xo")
                    nc.sync.dma_start(
                        xo[:], x_own[m * P:(m + 1) * P, n * 512:(n + 1) * 512])
                    nc.vector.tensor_tensor(h1[:, m, n * 512:(n + 1) * 512],
                                            ps_o[:], xo[:], op=OP.add)
                    sqc = pc.tile([P, 512], F32, tag="sqc", name="sqc")
                    nc.scalar.activation(sqc[:], h1[:, m, n * 512:(n + 1) * 512],
                                         AF.Square,
                                         accum_out=v2acc[:, m, n:n + 1])

        attp_cm.__exit__(None, None, None)
        hnp_cm = tc.tile_pool(name="hnp", bufs=1, side="right")
        hnp = hnp_cm.__enter__()
        hnT = hnp.tile([P, DB, TOK], BF)

        # ------------- Phase D1: hnT (rmsnorm of h1, transposed) -------------
        with (
            tc.tile_pool(name="pd1s", bufs=1) as pd1s,
            tc.tile_pool(name="pdps", bufs=2, space="PSUM") as pdps,
        ):
            for m in range(TB):
                v2 = pd1s.tile([P, 1], F32, tag="v2", name="v2")
                nc.vector.tensor_reduce(v2[:], v2acc[:, m, :],
                                        axis=mybir.AxisListType.X, op=OP.add)
                t2 = pd1s.tile([P, 1], F32, tag="t2", name="t2")
                nc.vector.tensor_scalar(t2[:], v2[:], 1.0 / D, EPS,
                                        op0=OP.mult, op1=OP.add)
                r2 = pd1s.tile([P, 1], F32, tag="r2", name="r2")
                nc.vector.reciprocal(r2[:], t2[:])
                s2 = pd1s.tile([P, 1], F32, tag="s2", name="s2")
                nc.scalar.activation(s2[:], r2[:], AF.Sqrt)
                hn = pd1s.tile([P, D], BF, tag="hn", name="hn")
                nc.vector.tensor_scalar(hn[:], h1[:, m, :], s2[:], None,
                                        op0=OP.mult)
                for j in range(DB):
                    ps_t = pdps.tile([P, P], BF, tag="ps_tr", name="ps_t")
                    nc.tensor.transpose(ps_t[:], hn[:, j * P:(j + 1) * P],
                                        ident[:])
                    nc.scalar.copy(hnT[:, j, m * P:(m + 1) * P], ps_t[:])

        h1p_cm.__exit__(None, None, None)
        wcp_cm = tc.tile_pool(name="wcp", bufs=1, side="left")
        wcp = wcp_cm.__enter__()
        vh = wcp.tile([P, RB, TOK], F32)       # scaled state vhat, 64KB/p
        clamT = wcp.tile([P, RB, TOK], BF)     # 32KB/p
        aT8 = wcp.tile([P, RB, TOK], F8, tag="aTs")
        yT8 = wcp.tile([P, DB, TOK], F8, tag="yTs")
        diag8 = wcp.tile([P, RB, P], F8)       # 4KB/p
        diagb = wcp.tile([P, RB, P], BF)       # 8KB/p
        identc = wcp.tile([P, NSTEPS - 1, P], BF)  # 2.25KB/p

        # ------------- Phase D2: b-proj -> clamT + vh init; diag/identc -------------
        with (
            tc.tile_pool(name="pd3s", bufs=2) as pd3s,
            tc.tile_pool(name="pd3ps", bufs=2, space="PSUM") as pd3ps,
        ):
            for r in range(RB):
                wn_sb = pd3s.tile([P, DB, P], BF, tag="wn", name="wn_sb")
                nc.sync.dma_start(wn_sb[:], wlcan_r[r])
                ps_b = pd3ps.tile([P, TOK], F32, tag="ps_b", name="ps_b")
                for j in range(DB):
                    nc.tensor.matmul(ps_b[:], wn_sb[:, j, :], hnT[:, j, :],
                                     start=(j == 0), stop=(j == DB - 1))
                nc.scalar.activation(clamT[:, r, :], ps_b[:], AF.Identity,
                                     scale=0.1, bias=bias_clam[:])
                nc.scalar.activation(vh[:, r, :], ps_b[:], AF.Identity,
                                     scale=0.1 * SSC, bias=bias_vinit[:])
            gst = pd3s.tile([P, RB], F32, tag="gst", name="gst")
            nc.sync.dma_start(gst[:], gs64[:])
            for r in range(RB):
                nc.vector.tensor_scalar(diag8[:, r, :], ident[:],
                                        gst[:, r:r + 1], None, op0=OP.mult)
                nc.vector.tensor_scalar(diagb[:, r, :], ident[:],
                                        gst[:, r:r + 1], None, op0=OP.mult)
            for t in range(1, NSTEPS):
                nc.vector.tensor_scalar(identc[:, t - 1, :], ident[:],
                                        SSC * 0.9 ** (-t), None, op0=OP.mult)

        hnp_cm.__exit__(None, None, None)

        # ---------------- Phase E: LCA recurrence ----------------
        NFP8 = NSTEPS - 1 - BF_LAST
        with (
            tc.tile_pool(name="pe", bufs=3) as pe,
            tc.tile_pool(name="peb", bufs=2) as peb,
            tc.tile_pool(name="peps", bufs=2, space="PSUM") as peps,
        ):
            K2Y = DB            # 16 pair-tiles contracting DLCA
            K2Z = DB // 2       # 8 pair-tiles contracting D

            for t in range(1, NSTEPS):
                if t <= NFP8:
                    for r in range(RB):
                        nc.scalar.activation(aT8[:, r, :], vh[:, r, :], AF.Relu,
                                             scale=1.0 / SSC)
                    for d in range(DB):
                        w1_sb = pe.tile([P, K2Y, 2, P], F8, tag="w1",
                                        name="w1_sb")
                        nc.sync.dma_start(w1_sb[:], wlcats8[d])
                        ps_y = peps.tile([P, TOK], F32, tag="ps_y", name="ps_y")
                        for k2 in range(K2Y):
                            nc.tensor.matmul(ps_y[:], w1_sb[:, k2, :, :],
                                             aT8[:, 2 * k2:2 * k2 + 2, :],
                                             start=(k2 == 0),
                                             stop=(k2 == K2Y - 1), perf_mode=DR)
                        nc.scalar.activation(yT8[:, d, :], ps_y[:], AF.Identity,
                                             scale=SMID)
                    for r in range(RB):
                        w2_sb = pe.tile([P, K2Z, 2, P], F8, tag="w2s",
                                        name="w2_sb")
                        nc.sync.dma_start(w2_sb[:], wlca8[r])
                        ps_z = peps.tile([P, TOK], F32, tag="ps_z", name="ps_z")
                        for j2 in range(K2Z):
                            nc.tensor.matmul(ps_z[:], w2_sb[:, j2, :, :],
                                             yT8[:, 2 * j2:2 * j2 + 2, :],
                                             start=(j2 == 0), stop=False,
                                             perf_mode=DR)
                        nc.tensor.matmul(ps_z[:], diag8[:, r, :], aT8[:, r, :],
                                         start=False, stop=False)
                        nc.tensor.matmul(ps_z[:], identc[:, t - 1, :],
                                         clamT[:, r, :], start=False, stop=True)
                        nc.vector.tensor_tensor(vh[:, r, :], vh[:, r, :],
                                                ps_z[:], op=OP.add)
                else:
                    aTb = wcp.tile([P, RB, TOK], BF, tag="aTs", name="aTb")
                    yTb = wcp.tile([P, DB, TOK], BF, tag="yTs", name="yTb")
                    for r in range(RB):
                        nc.scalar.activation(aTb[:, r, :], vh[:, r, :], AF.Relu,
                                             scale=1.0 / SSC)
                    for d in range(DB):
                        w1b = peb.tile([P, RB, P], BF, tag="w1b", name="w1b")
                        nc.sync.dma_start(w1b[:], wlcatsb[d])
                        ps_y = peps.tile([P, TOK], F32, tag="ps_y", name="ps_y")
                        for k in range(RB):
                            nc.tensor.matmul(ps_y[:], w1b[:, k, :],
                                             aTb[:, k, :], start=(k == 0),
                                             stop=(k == RB - 1))
                        nc.scalar.activation(yTb[:, d, :], ps_y[:], AF.Identity,
                                             scale=SMID)
                    for r in range(RB):
                        w2b = peb.tile([P, DB, P], BF, tag="w2b", name="w2b")
                        nc.sync.dma_start(w2b[:], wlcab[r])
                        ps_z = peps.tile([P, TOK], F32, tag="ps_z", name="ps_z")
                        for j in range(DB):
                            nc.tensor.matmul(ps_z[:], w2b[:, j, :], yTb[:, j, :],
                                             start=(j == 0), stop=False)
                        nc.tensor.matmul(ps_z[:], diagb[:, r, :], aTb[:, r, :],
                                         start=False, stop=False)
                        nc.tensor.matmul(ps_z[:], identc[:, t - 1, :],
                                         clamT[:, r, :], start=False, stop=True)
                        nc.vector.tensor_tensor(vh[:, r, :], vh[:, r, :],
                                                ps_z[:], op=OP.add)

        # final rho in bf16 for the output projection
        atp_cm = tc.tile_pool(name="atp", bufs=1, side="right")
        atp = atp_cm.__enter__()
        aTf = atp.tile([P, RB, TOK], BF)
        for r in range(RB):
            nc.scalar.activation(aTf[:, r, :], vh[:, r, :], AF.Relu,
                                 scale=1.0 / SSC)

        wcp_cm.__exit__(None, None, None)
        h2p_cm = tc.tile_pool(name="h2p", bufs=1, side="left")
        h2p = h2p_cm.__enter__()
        h2 = h2p.tile([P, TB, D], F32)
        v3acc = h2p.tile([P, TB, 4], F32)

        # ---------------- Phase F: h2 = 0.9^9 * rho @ W_lca^T ----------------
        with (
            tc.tile_pool(name="pf", bufs=2) as pf,
            tc.tile_pool(name="pfps", bufs=2, space="PSUM") as pfps,
        ):
            for n in range(4):
                wt_sb = pf.tile([P, RB, 512], BF, tag="wts", name="wt_sb")
                nc.sync.dma_start(wt_sb[:], wlcats_n[n])
                for m in range(TB):
                    ps_h = pfps.tile([P, 512], F32, tag="ps_h", name="ps_h")
                    for k in range(RB):
                        nc.tensor.matmul(ps_h[:], aTf[:, k, m * P:(m + 1) * P],
                                         wt_sb[:, k, :], start=(k == 0),
                                         stop=(k == RB - 1))
                    nc.scalar.copy(h2[:, m, n * 512:(n + 1) * 512], ps_h[:])
                    sqf = pf.tile([P, 512], F32, tag="sqf", name="sqf")
                    nc.scalar.activation(sqf[:], ps_h[:], AF.Square,
                                         accum_out=v3acc[:, m, n:n + 1])

        atp_cm.__exit__(None, None, None)

        # ---------------- Phase G: MLP ----------------
        with (
            tc.tile_pool(name="pg", bufs=1, side="right") as pg,
            tc.tile_pool(name="pgs1", bufs=1) as pgs1,
            tc.tile_pool(name="pgs", bufs=2) as pgs,
            tc.tile_pool(name="pgps", bufs=2, space="PSUM") as pgps,
            tc.tile_pool(name="pgpd", bufs=1, space="PSUM") as pgpd,
        ):
            prodT = pg.tile([P, FB, TOK], BF)      # 64KB/p
            mT = pg.tile([P, DB, TOK], BF)
            for m in range(TB):
                v3 = pgs1.tile([P, 1], F32, tag="v3", name="v3")
                nc.vector.tensor_reduce(v3[:], v3acc[:, m, :],
                                        axis=mybir.AxisListType.X, op=OP.add)
                t3 = pgs1.tile([P, 1], F32, tag="t3", name="t3")
                nc.vector.tensor_scalar(t3[:], v3[:], 1.0 / D, EPS,
                                        op0=OP.mult, op1=OP.add)
                r3 = pgs1.tile([P, 1], F32, tag="r3", name="r3")
                nc.vector.reciprocal(r3[:], t3[:])
                s3 = pgs1.tile([P, 1], F32, tag="s3", name="s3")
                nc.scalar.activation(s3[:], r3[:], AF.Sqrt)
                mb = pgs1.tile([P, D], BF, tag="mb", name="mb")
                nc.vector.tensor_scalar(mb[:], h2[:, m, :], s3[:], None,
                                        op0=OP.mult)
                for j in range(DB):
                    ps_t = pgps.tile([P, P], BF, tag="ps_tr3", name="ps_t")
                    nc.tensor.transpose(ps_t[:], mb[:, j * P:(j + 1) * P],
                                        ident[:])
                    nc.scalar.copy(mT[:, j, m * P:(m + 1) * P], ps_t[:])

            for f in range(FB):
                wgs = pgs.tile([P, DB, HD], BF, tag="wgs", name="wgs")
                nc.sync.dma_start(wgs[:], wg_r[f])
                ps_g = pgps.tile([P, TOK], F32, tag="ps_g", name="ps_g")
                for j in range(DB):
                    nc.tensor.matmul(ps_g[:], wgs[:, j, :], mT[:, j, :],
                                     start=(j == 0), stop=(j == DB - 1))
                gT = pgs.tile([P, TOK], BF, tag="gT", name="gT")
                nc.scalar.activation(gT[:], ps_g[:], AF.Silu)
                wus = pgs.tile([P, DB, HD], BF, tag="wus", name="wus")
                nc.sync.dma_start(wus[:], wu_r[f])
                ps_u = pgps.tile([P, TOK], F32, tag="ps_g", name="ps_u")
                for j in range(DB):
                    nc.tensor.matmul(ps_u[:], wus[:, j, :], mT[:, j, :],
                                     start=(j == 0), stop=(j == DB - 1))
                nc.vector.tensor_tensor(prodT[:, f, :], ps_u[:], gT[:],
                                        op=OP.mult)

            for n in range(4):
                ps_d = [pgpd.tile([P, 512], F32, tag=f"ps_d{m}",
                                  name=f"ps_d{m}")
                        for m in range(TB)]
                for kg in range(8):
                    wds = pgs.tile([P, 8, 512], BF, tag="wds", name="wds")
                    nc.sync.dma_start(wds[:], wd_n[n][kg])
                    for m in range(TB):
                        for k in range(8):
                            kk = kg * 8 + k
                            nc.tensor.matmul(
                                ps_d[m][:], prodT[:, kk, m * P:(m + 1) * P],
                                wds[:, k, :], start=(kg == 0 and k == 0),
                                stop=(kg == 7 and k == 7))
                for m in range(TB):
                    yo = pgs.tile([P, 512], F32, tag="yo", name="yo")
                    nc.vector.tensor_tensor(yo[:], ps_d[m][:],
                                            h2[:, m, n * 512:(n + 1) * 512],
                                            op=OP.add)
                    nc.sync.dma_start(
                        y[m * P:(m + 1) * P, n * 512:(n + 1) * 512], yo[:])

        h2p_cm.__exit__(None, None, None)

    nc.compile()
    return nc


_NC_CACHE = None


def _get_nc():
    global _NC_CACHE
    if _NC_CACHE is None:
        _NC_CACHE = build_nc()
    return _NC_CACHE


def _prep_weights(inputs):
    f32 = np.float32
    wln_in = np.asarray(inputs["w_ln_in"], f32)
    wln_lca = np.asarray(inputs["w_ln_lca"], f32)
    wln_post = np.asarray(inputs["w_ln_post"], f32)
    Wq = np.asarray(inputs["Wq"], f32) * wln_in[:, None]
    Wk = np.asarray(inputs["Wk"], f32) * wln_in[:, None]
    Wv = np.asarray(inputs["Wv"], f32) * wln_in[:, None]
    Wo = np.asarray(inputs["Wo"], f32)
    Wlca = np.asarray(inputs["W_lca"], f32)
    Wlca_n = Wlca * wln_lca[:, None]
    Wg = np.asarray(inputs["W_gate"], f32) * wln_post[:, None]
    Wu = np.asarray(inputs["W_up"], f32) * wln_post[:, None]
    Wd = np.asarray(inputs["W_down"], f32)
    WlcaT = np.ascontiguousarray(Wlca.T)
    gs = np.einsum("ij,ij->j", Wlca, Wlca)
    gs64 = np.ascontiguousarray((64.0 * gs).reshape(RB, P).T).astype(f32)
    c = lambda a: np.ascontiguousarray(a).astype(bf16)
    return {
        "wq_r": c(_swz(Wq, H, HD)), "wk_r": c(_swz(Wk, H, HD)),
        "wv_g": c(_swz(Wv, 4, 512)), "wo_n": c(_swz(Wo, 4, 512)),
        "wlcan_r": c(_swz(Wlca_n, RB, P)),
        "wlcats8": _swz_dr(WlcaT, DB, P, -A1, fp8),
        "wlca8": _swz_dr(Wlca, RB, P, A2, fp8),
        "wlcatsb": c(_swz(WlcaT * (-A1), DB, P)),
        "wlcab": c(_swz(Wlca * A2, RB, P)),
        "gs64": gs64,
        "wlcats_n": c(_swz(WlcaT * DEC9, 4, 512)),
        "wg_r": c(_swz(Wg, FB, HD)), "wu_r": c(_swz(Wu, FB, HD)),
        "wd_n": np.ascontiguousarray(
            c(_swz(Wd, 4, 512)).reshape(4, P, 8, 8, 512)
            .transpose(0, 2, 1, 3, 4)),
    }


def make_in_maps(inputs):
    hs = np.asarray(inputs["hidden_states"], np.float32).reshape(B * S, D)
    wmaps = _prep_weights(inputs)
    cos, sin = _rope_tables()
    all_pos = np.concatenate([_core_token_map(r) for r in range(NCORE)])
    kv_pos, kv_batch = all_pos % S, all_pos // S
    in_maps, owns = [], []
    for cix in range(NCORE):
        own = _core_token_map(cix)
        x_own = np.ascontiguousarray(hs[own])
        xoT = np.ascontiguousarray(x_own.T).astype(bf16)
        q_pos, q_batch = own % S, own // S
        vis = (kv_batch[:, None] == q_batch[None, :]) & (
            kv_pos[:, None] <= q_pos[None, :])
        vis = vis.astype(np.float32).reshape(KVB2, P, TOK)
        mhalf = np.zeros((KVB2, P, 256), np.float32)
        for t in range(KVB2):
            q0 = ((t % 4) // 2) * 256
            mhalf[t] = vis[t, :, q0:q0 + 256]
        maskT = np.ascontiguousarray(mhalf.transpose(1, 0, 2)).astype(bf16)
        cosT = np.ascontiguousarray(cos[q_pos].T).astype(bf16)
        sinT = np.ascontiguousarray(sin[q_pos].T)
        sinT[:HD // 2] *= -1.0
        sinT = sinT.astype(bf16)
        m = {
            "x_own": x_own, "xoT": xoT,
            "maskT": maskT, "cosT": cosT, "sinT": sinT, **wmaps,
        }
        in_maps.append(m)
        owns.append(own)
    return in_maps, owns


def kernel(**inputs) -> np.ndarray:
    nc = _get_nc()
    in_maps, owns = make_in_maps(inputs)
    res = run_bass_kernel_spmd(nc, in_maps, core_ids=list(range(NCORE)))
    out = np.zeros((B * S, D), np.float32)
    for cix in range(NCORE):
        out[owns[cix]] = res.results[cix]["y"]
    return out.reshape(B, S, D)
